# revision 2
# baseline (speedup 1.0000x reference)
"""Two-layer LSTM (H=51) over [B=4096, T=256] on 8 NeuronCores — v14 (fp16, G4, 128-col lhsT).

The phase period of this kernel is the single-group dependency chain
(matmuls -> gate activations -> c update -> tanh(c) -> h update -> matmuls),
so v5 minimizes chain latency:
- All-sigmoid formulation: ONE ACT op per group covers all four gate
  banks with Sigmoid; the g-bank weights are pre-doubled so
  tanh(zg) = 2*sigmoid(2*zg) - 1 is reconstructed on DVE with a cheap
  4x-mode tensor_scalar. Cell updates are plain bf16 tensor_tensor:
  v = si*tgx, u = sf*c, c = u+v, h = so*tanh(c). Only two ACT ops per
  group per phase (gates sigmoid + tanh(c)) minimizes both ACT busy time
  and cross-group queueing on the in-order ACT engine.
Carried over from v2-v4: merged l1+l2 gate matmuls, x via staged DMA ring +
K=1 rank-1 matmuls, bf16 weights/states, skewed T+2-phase pipeline, head
matmul batched in PSUM and flushed every 128 steps.
"""

import numpy as np

H = 51
T_FULL = 256
B_FULL = 4096
N_CORES = 8

ROW_H1 = 0
ROW_H2 = 64
ROW_ONES = 115
K_STK = 116
GP = 115
MW = GP      # live lhsT columns per bank
MWP = 128    # padded bank stride (NumWeights==128 enables FWL on hw)

BANKS = ["i", "f", "o", "g"]  # PSUM bank order


def _bank_cols(xi):
    return slice(xi * MWP, (xi + 1) * MWP)


def _build_weights(W_ih1, W_hh1, b_ih1, b_hh1, W_ih2, W_hh2, b_ih2, b_hh2,
                   W_lin, b_lin):
    """lhsT packing; no gate scaling (sigmoid formulation, plain states).

    WG [K_STK, 12*MW + 1]: per bank xi in order i,f,o,g: combined l1+l2
    lhsT at xi, l1-only at 4+xi, l2-only at 8+xi; head at col 12*MW.
    WX [1, 4*MW]: K=1 x-weight lhsT per bank.
    """
    b1 = (b_ih1 + b_hh1).astype(np.float64)
    b2 = (b_ih2 + b_hh2).astype(np.float64)
    idx = {"i": np.arange(0, H), "f": np.arange(H, 2 * H),
           "g": np.arange(2 * H, 3 * H), "o": np.arange(3 * H, 4 * H)}
    WG = np.zeros((K_STK, 12 * MWP + 1), dtype=np.float64)
    WXc = np.zeros((4 * MWP,), dtype=np.float64)
    for xi, gate in enumerate(BANKS):
        r = idx[gate]
        s = 2.0 if gate == "g" else 1.0  # tanh(z) = 2*sigmoid(2z) - 1
        l1col = slice(xi * MWP, xi * MWP + H)
        WG[ROW_ONES, l1col] = s * b1[r]
        WG[ROW_H1:ROW_H1 + H, l1col] = s * W_hh1[r, :].T
        l2col = slice(xi * MWP + ROW_H2, xi * MWP + ROW_H2 + H)
        WG[ROW_ONES, l2col] = s * b2[r]
        WG[ROW_H1:ROW_H1 + H, l2col] = s * W_ih2[r, :].T
        WG[ROW_H2:ROW_H2 + H, l2col] = s * W_hh2[r, :].T
        WXc[xi * MWP:xi * MWP + H] = s * W_ih1[r, 0]
    for xi in range(4):
        comb = WG[:, xi * MWP:(xi + 1) * MWP]
        l1 = np.array(comb)
        l1[:, ROW_H2:ROW_H2 + H] = 0.0
        l2 = np.array(comb)
        l2[:, 0:H] = 0.0
        WG[:, (4 + xi) * MWP:(5 + xi) * MWP] = l1
        WG[:, (8 + xi) * MWP:(9 + xi) * MWP] = l2
    WG[ROW_H2:ROW_H2 + H, 12 * MWP] = W_lin[0, :]
    WG[ROW_ONES, 12 * MWP] = float(np.asarray(b_lin).reshape(-1)[0])
    WX = WXc[None, :]
    return WG, WX


def build_core_kernel(T, B, groups=2, ct_f32=False):
    import concourse.bacc as bacc
    import concourse.mybir as mybir
    from concourse.tile import TileContext

    fp = mybir.dt.float32
    bf = mybir.dt.float16
    fc = fp if ct_f32 else bf
    Bg = B // groups
    NB = 6

    nc = bacc.Bacc("TRN2", target_bir_lowering=False, debug=False)
    xT = nc.dram_tensor("xT", [T + 1, B], bf, kind="ExternalInput")
    WG = nc.dram_tensor("WG", [K_STK, 12 * MWP + 1], bf, kind="ExternalInput")
    WX = nc.dram_tensor("WX", [1, 4 * MWP], bf, kind="ExternalInput")
    out_bt = nc.dram_tensor("out_bt", [B, T], fp, kind="ExternalOutput")

    C = min(128, T)
    assert T % C == 0
    assert Bg % 128 == 0

    with TileContext(nc) as tc:
        with (
            tc.tile_pool(name="persist", bufs=1) as persist,
            tc.tile_pool(name="gpsum", bufs=1, space="PSUM") as gpsum,
            tc.tile_pool(name="opsum", bufs=1, space="PSUM") as opsum,
            tc.tile_pool(name="temps", bufs=3) as temps,
            tc.tile_pool(name="ostage", bufs=2) as ostage,
            tc.tile_pool(name="xring", bufs=NB) as xring,
        ):
            wg = persist.tile([K_STK, 12 * MWP + 1], bf)
            nc.sync.dma_start(out=wg, in_=WG[:, :])
            wx = persist.tile([1, 4 * MWP], bf)
            nc.sync.dma_start(out=wx, in_=WX[:, :])
            xsts = []
            for q in range(T):
                xst = xring.tile([1, B], bf, tag="xst")
                nc.sync.dma_start(out=xst, in_=xT[q + 1:q + 2, :])
                xsts.append(xst)

            nchunk = Bg // 128
            stks, gps, pos, cts = [], [], [], []
            for g in range(groups):
                ctt = persist.tile([GP, Bg], fc, tag=f"ct{g}", name=f"ctt{g}")
                nc.vector.memset(ctt[:, :], 0.0)
                cts.append(ctt[:, :])
            for g in range(groups):
                stk = persist.tile([K_STK, Bg], bf, tag=f"stk{g}")
                gp = gpsum.tile([128, 4 * Bg], fp, tag=f"gp{g}")
                nc.vector.memset(stk[:, :], 0.0)
                nc.sync.dma_start(out=stk[ROW_ONES:ROW_ONES + 1, :],
                                  in_=xT[0:1, g * Bg:(g + 1) * Bg])
                stks.append(stk)
                gps.append(gp)
                pos.append(opsum.tile([128, nchunk * C], fp, tag=f"po{g}",
                                      name=f"po{g}"))

            add = mybir.AluOpType.add
            mult = mybir.AluOpType.mult
            tanh = mybir.ActivationFunctionType.Tanh
            sigm = mybir.ActivationFunctionType.Sigmoid

            for q in range(T + 2):
                l1 = q < T
                l2 = 1 <= q <= T
                if l1:
                    for g in range(groups):
                        cols = slice(g * Bg, (g + 1) * Bg)
                        for xi in range(4):
                            nc.tensor.matmul(
                                gps[g][0:128, xi * Bg:(xi + 1) * Bg],
                                wx[0:1, xi * MWP:(xi + 1) * MWP],
                                xsts[q][0:1, cols],
                                start=True, stop=False)
                for g in range(groups):
                    rhs = stks[g][0:K_STK, :]
                    for xi in range(4):
                        reg = gps[g][0:128, xi * Bg:(xi + 1) * Bg]
                        if l1 and l2:
                            nc.tensor.matmul(
                                reg, wg[0:K_STK, _bank_cols(xi)], rhs,
                                start=False, stop=True)
                        elif l1:
                            nc.tensor.matmul(
                                reg, wg[0:K_STK, _bank_cols(4 + xi)], rhs,
                                start=False, stop=True)
                        elif l2:
                            nc.tensor.matmul(
                                reg, wg[0:K_STK, _bank_cols(8 + xi)], rhs,
                                start=True, stop=True)
                if q >= 2:
                    t = q - 2
                    tc_col = t % C
                    for g in range(groups):
                        for k in range(nchunk):
                            nc.tensor.matmul(
                                pos[g][:, k * C + tc_col:k * C + tc_col + 1],
                                stks[g][64:116, k * 128:(k + 1) * 128],
                                wg[64:116, 12 * MWP:12 * MWP + 1],
                                start=True, stop=True)
                    if tc_col == C - 1:
                        t0 = t - (C - 1)
                        for g in range(groups):
                            for k in range(nchunk):
                                st = ostage.tile([128, C], fp, tag=f"os{g}_{k}")
                                nc.scalar.copy(st, pos[g][:, k * C:(k + 1) * C])
                                row0 = g * Bg + k * 128
                                nc.sync.dma_start(
                                    out=out_bt[row0:row0 + 128, t0:t0 + C],
                                    in_=st)
                if q > T:
                    continue
                # ACT: one sigmoid op over all four banks
                sg_ts = []
                for g in range(groups):
                    sg_t = temps.tile([GP, 4 * Bg], bf, tag=f"sg{g}")
                    nc.scalar.activation(sg_t[:, :], gps[g][0:GP, :], sigm)
                    sg_ts.append(sg_t)
                # DVE: tgx = 2*sg - 1 ; u = sf*c ; v = si*tgx ; c = u + v
                for g in range(groups):
                    sg_t, ct = sg_ts[g], cts[g]

                    si = sg_t[:, 0 * Bg:1 * Bg]
                    sf = sg_t[:, 1 * Bg:2 * Bg]
                    sg = sg_t[:, 3 * Bg:4 * Bg]
                    tgx = temps.tile([GP, Bg], bf, tag=f"tgx{g}")
                    u = temps.tile([GP, Bg], fc, tag=f"u{g}")
                    v = temps.tile([GP, Bg], bf, tag=f"v{g}")
                    nc.vector.tensor_scalar(tgx, sg, 2.0, 1.0, mult,
                                            mybir.AluOpType.subtract)
                    nc.vector.tensor_tensor(u, sf, ct, mult)
                    nc.vector.tensor_tensor(v, si, tgx, mult)
                    nc.vector.tensor_tensor(ct, u, v, add)
                # ACT: tanh(c)
                tcls = []
                for g in range(groups):
                    tcl = temps.tile([GP, Bg], bf, tag=f"tc{g}")
                    nc.scalar.activation(tcl, cts[g], tanh)
                    tcls.append(tcl)
                # DVE: h = so * tanh(c)
                for g in range(groups):
                    so = sg_ts[g][:, 2 * Bg:3 * Bg]
                    nc.vector.tensor_tensor(
                        stks[g][ROW_H1:ROW_H1 + GP, :], so, tcls[g], mult)
    nc.compile()
    return nc


_NC_CACHE = {}


def _get_nc(T, B, groups=2, ct_f32=False):
    key = (T, B, groups, ct_f32)
    if key not in _NC_CACHE:
        _NC_CACHE[key] = build_core_kernel(T, B, groups, ct_f32)
    return _NC_CACHE[key]


def kernel(input, W_ih1, W_hh1, b_ih1, b_hh1, W_ih2, W_hh2, b_ih2, b_hh2,
           W_lin, b_lin, _groups=4, _ct_f32=0):
    from concourse import bass_utils

    bf = np.float16
    input = np.asarray(input, dtype=np.float32)
    B, T = input.shape
    Bc = B // N_CORES
    WG, WX = _build_weights(
        np.asarray(W_ih1, np.float64), np.asarray(W_hh1, np.float64),
        np.asarray(b_ih1, np.float64), np.asarray(b_hh1, np.float64),
        np.asarray(W_ih2, np.float64), np.asarray(W_hh2, np.float64),
        np.asarray(b_ih2, np.float64), np.asarray(b_hh2, np.float64),
        np.asarray(W_lin, np.float64), np.asarray(b_lin, np.float64))
    WG = WG.astype(bf)
    WX = WX.astype(bf)
    xT = np.concatenate([np.ones((1, B), np.float32),
                         input.T.astype(np.float32)]).astype(bf)
    nc = _get_nc(T, Bc, _groups, bool(_ct_f32))
    in_maps = [
        {"xT": np.ascontiguousarray(xT[:, c * Bc:(c + 1) * Bc]),
         "WG": WG, "WX": WX}
        for c in range(N_CORES)
    ]
    res = bass_utils.run_bass_kernel_spmd(
        nc, in_maps, core_ids=list(range(N_CORES)), trace=False)
    outs = [res.results[c]["out_bt"] for c in range(N_CORES)]
    out = np.concatenate(outs, axis=0)
    return out.astype(np.float32)


# revision 3
# speedup vs baseline: 1.0023x; 1.0023x over previous
"""Two-layer LSTM (H=51) over [B=4096, T=256] on 8 NeuronCores — v14 (fp16, G4, 128-col lhsT).

The phase period of this kernel is the single-group dependency chain
(matmuls -> gate activations -> c update -> tanh(c) -> h update -> matmuls),
so v5 minimizes chain latency:
- All-sigmoid formulation: ONE ACT op per group covers all four gate
  banks with Sigmoid; the g-bank weights are pre-doubled so
  tanh(zg) = 2*sigmoid(2*zg) - 1 is reconstructed on DVE with a cheap
  4x-mode tensor_scalar. Cell updates are plain bf16 tensor_tensor:
  v = si*tgx, u = sf*c, c = u+v, h = so*tanh(c). Only two ACT ops per
  group per phase (gates sigmoid + tanh(c)) minimizes both ACT busy time
  and cross-group queueing on the in-order ACT engine.
Carried over from v2-v4: merged l1+l2 gate matmuls, x via staged DMA ring +
K=1 rank-1 matmuls, bf16 weights/states, skewed T+2-phase pipeline, head
matmul batched in PSUM and flushed every 128 steps.
"""

import numpy as np

H = 51
T_FULL = 256
B_FULL = 4096
N_CORES = 8

ROW_H1 = 0
ROW_H2 = 64
ROW_ONES = 115
K_STK = 116
GP = 115
MW = GP      # live lhsT columns per bank
MWP = 128    # padded bank stride (NumWeights==128 enables FWL on hw)

BANKS = ["i", "f", "o", "g"]  # PSUM bank order


def _bank_cols(xi):
    return slice(xi * MWP, (xi + 1) * MWP)


def _build_weights(W_ih1, W_hh1, b_ih1, b_hh1, W_ih2, W_hh2, b_ih2, b_hh2,
                   W_lin, b_lin):
    """lhsT packing; no gate scaling (sigmoid formulation, plain states).

    WG [K_STK, 12*MW + 1]: per bank xi in order i,f,o,g: combined l1+l2
    lhsT at xi, l1-only at 4+xi, l2-only at 8+xi; head at col 12*MW.
    WX [1, 4*MW]: K=1 x-weight lhsT per bank.
    """
    b1 = (b_ih1 + b_hh1).astype(np.float64)
    b2 = (b_ih2 + b_hh2).astype(np.float64)
    idx = {"i": np.arange(0, H), "f": np.arange(H, 2 * H),
           "g": np.arange(2 * H, 3 * H), "o": np.arange(3 * H, 4 * H)}
    WG = np.zeros((K_STK, 12 * MWP + 1), dtype=np.float64)
    WXc = np.zeros((4 * MWP,), dtype=np.float64)
    for xi, gate in enumerate(BANKS):
        r = idx[gate]
        s = 2.0 if gate == "g" else 1.0  # tanh(z) = 2*sigmoid(2z) - 1
        l1col = slice(xi * MWP, xi * MWP + H)
        WG[ROW_ONES, l1col] = s * b1[r]
        WG[ROW_H1:ROW_H1 + H, l1col] = s * W_hh1[r, :].T
        l2col = slice(xi * MWP + ROW_H2, xi * MWP + ROW_H2 + H)
        WG[ROW_ONES, l2col] = s * b2[r]
        WG[ROW_H1:ROW_H1 + H, l2col] = s * W_ih2[r, :].T
        WG[ROW_H2:ROW_H2 + H, l2col] = s * W_hh2[r, :].T
        WXc[xi * MWP:xi * MWP + H] = s * W_ih1[r, 0]
    for xi in range(4):
        comb = WG[:, xi * MWP:(xi + 1) * MWP]
        l1 = np.array(comb)
        l1[:, ROW_H2:ROW_H2 + H] = 0.0
        l2 = np.array(comb)
        l2[:, 0:H] = 0.0
        WG[:, (4 + xi) * MWP:(5 + xi) * MWP] = l1
        WG[:, (8 + xi) * MWP:(9 + xi) * MWP] = l2
    WG[ROW_H2:ROW_H2 + H, 12 * MWP] = W_lin[0, :]
    WG[ROW_ONES, 12 * MWP] = float(np.asarray(b_lin).reshape(-1)[0])
    WX = WXc[None, :]
    return WG, WX


def build_core_kernel(T, B, groups=2, ct_f32=False):
    import concourse.bacc as bacc
    import concourse.mybir as mybir
    from concourse.tile import TileContext

    fp = mybir.dt.float32
    bf = mybir.dt.float16
    fc = fp if ct_f32 else bf
    Bg = B // groups
    NB = 4

    nc = bacc.Bacc("TRN2", target_bir_lowering=False, debug=False)
    xT = nc.dram_tensor("xT", [T + 1, B], bf, kind="ExternalInput")
    WG = nc.dram_tensor("WG", [K_STK, 12 * MWP + 1], bf, kind="ExternalInput")
    WX = nc.dram_tensor("WX", [1, 4 * MWP], bf, kind="ExternalInput")
    out_bt = nc.dram_tensor("out_bt", [B, T], fp, kind="ExternalOutput")

    C = min(128, T)
    assert T % C == 0
    assert Bg % 128 == 0

    with TileContext(nc) as tc:
        with (
            tc.tile_pool(name="persist", bufs=1) as persist,
            tc.tile_pool(name="gpsum", bufs=1, space="PSUM") as gpsum,
            tc.tile_pool(name="opsum", bufs=1, space="PSUM") as opsum,
            tc.tile_pool(name="temps", bufs=3) as temps,
            tc.tile_pool(name="ostage", bufs=2) as ostage,
            tc.tile_pool(name="xring", bufs=NB) as xring,
        ):
            wg = persist.tile([K_STK, 12 * MWP + 1], bf)
            nc.sync.dma_start(out=wg, in_=WG[:, :])
            wx = persist.tile([1, 4 * MWP], bf)
            nc.sync.dma_start(out=wx, in_=WX[:, :])
            xsts = []
            for q in range(T):
                xst = xring.tile([1, B], bf, tag="xst")
                nc.sync.dma_start(out=xst, in_=xT[q + 1:q + 2, :])
                xsts.append(xst)

            nchunk = Bg // 128
            stks, gps, pos, cts = [], [], [], []
            for g in range(groups):
                ctt = persist.tile([GP, Bg], fc, tag=f"ct{g}", name=f"ctt{g}")
                nc.vector.memset(ctt[:, :], 0.0)
                cts.append(ctt[:, :])
            for g in range(groups):
                stk = persist.tile([K_STK, Bg], bf, tag=f"stk{g}")
                gp = gpsum.tile([128, 4 * Bg], fp, tag=f"gp{g}")
                nc.vector.memset(stk[:, :], 0.0)
                nc.sync.dma_start(out=stk[ROW_ONES:ROW_ONES + 1, :],
                                  in_=xT[0:1, g * Bg:(g + 1) * Bg])
                stks.append(stk)
                gps.append(gp)
                pos.append(opsum.tile([128, nchunk * C], fp, tag=f"po{g}",
                                      name=f"po{g}"))

            add = mybir.AluOpType.add
            mult = mybir.AluOpType.mult
            tanh = mybir.ActivationFunctionType.Tanh
            sigm = mybir.ActivationFunctionType.Sigmoid

            for q in range(T + 2):
                l1 = q < T
                l2 = 1 <= q <= T
                if l1:
                    for g in range(groups):
                        cols = slice(g * Bg, (g + 1) * Bg)
                        for xi in range(4):
                            nc.tensor.matmul(
                                gps[g][0:128, xi * Bg:(xi + 1) * Bg],
                                wx[0:1, xi * MWP:(xi + 1) * MWP],
                                xsts[q][0:1, cols],
                                start=True, stop=False)
                for g in range(groups):
                    rhs = stks[g][0:K_STK, :]
                    for xi in range(4):
                        reg = gps[g][0:128, xi * Bg:(xi + 1) * Bg]
                        if l1 and l2:
                            nc.tensor.matmul(
                                reg, wg[0:K_STK, _bank_cols(xi)], rhs,
                                start=False, stop=True)
                        elif l1:
                            nc.tensor.matmul(
                                reg, wg[0:K_STK, _bank_cols(4 + xi)], rhs,
                                start=False, stop=True)
                        elif l2:
                            nc.tensor.matmul(
                                reg, wg[0:K_STK, _bank_cols(8 + xi)], rhs,
                                start=True, stop=True)
                if q >= 2:
                    t = q - 2
                    tc_col = t % C
                    for g in range(groups):
                        for k in range(nchunk):
                            nc.tensor.matmul(
                                pos[g][:, k * C + tc_col:k * C + tc_col + 1],
                                stks[g][64:116, k * 128:(k + 1) * 128],
                                wg[64:116, 12 * MWP:12 * MWP + 1],
                                start=True, stop=True)
                    if tc_col == C - 1:
                        t0 = t - (C - 1)
                        for g in range(groups):
                            for k in range(nchunk):
                                st = ostage.tile([128, C], fp, tag=f"os{g}_{k}")
                                nc.scalar.copy(st, pos[g][:, k * C:(k + 1) * C])
                                row0 = g * Bg + k * 128
                                nc.sync.dma_start(
                                    out=out_bt[row0:row0 + 128, t0:t0 + C],
                                    in_=st)
                if q > T:
                    continue
                # ACT: one sigmoid op over all four banks
                sg_ts = []
                for g in range(groups):
                    sg_t = temps.tile([GP, 4 * Bg], bf, tag=f"sg{g}")
                    nc.scalar.activation(sg_t[:, :], gps[g][0:GP, :], sigm)
                    sg_ts.append(sg_t)
                # DVE: tgx = 2*sg - 1 ; u = sf*c ; v = si*tgx ; c = u + v
                for g in range(groups):
                    sg_t, ct = sg_ts[g], cts[g]

                    si = sg_t[:, 0 * Bg:1 * Bg]
                    sf = sg_t[:, 1 * Bg:2 * Bg]
                    sg = sg_t[:, 3 * Bg:4 * Bg]
                    tgx = temps.tile([GP, Bg], bf, tag=f"tgx{g}")
                    u = temps.tile([GP, Bg], fc, tag=f"u{g}")
                    v = temps.tile([GP, Bg], bf, tag=f"v{g}")
                    nc.vector.tensor_scalar(tgx, sg, 2.0, 1.0, mult,
                                            mybir.AluOpType.subtract)
                    nc.vector.tensor_tensor(u, sf, ct, mult)
                    nc.vector.tensor_tensor(v, si, tgx, mult)
                    nc.vector.tensor_tensor(ct, u, v, add)
                # ACT: tanh(c)
                tcls = []
                for g in range(groups):
                    tcl = temps.tile([GP, Bg], bf, tag=f"tc{g}")
                    nc.scalar.activation(tcl, cts[g], tanh)
                    tcls.append(tcl)
                # DVE: h = so * tanh(c)
                for g in range(groups):
                    so = sg_ts[g][:, 2 * Bg:3 * Bg]
                    nc.vector.tensor_tensor(
                        stks[g][ROW_H1:ROW_H1 + GP, :], so, tcls[g], mult)
    nc.compile()
    return nc


_NC_CACHE = {}


def _get_nc(T, B, groups=2, ct_f32=False):
    key = (T, B, groups, ct_f32)
    if key not in _NC_CACHE:
        _NC_CACHE[key] = build_core_kernel(T, B, groups, ct_f32)
    return _NC_CACHE[key]


def kernel(input, W_ih1, W_hh1, b_ih1, b_hh1, W_ih2, W_hh2, b_ih2, b_hh2,
           W_lin, b_lin, _groups=4, _ct_f32=0):
    from concourse import bass_utils

    bf = np.float16
    input = np.asarray(input, dtype=np.float32)
    B, T = input.shape
    Bc = B // N_CORES
    WG, WX = _build_weights(
        np.asarray(W_ih1, np.float64), np.asarray(W_hh1, np.float64),
        np.asarray(b_ih1, np.float64), np.asarray(b_hh1, np.float64),
        np.asarray(W_ih2, np.float64), np.asarray(W_hh2, np.float64),
        np.asarray(b_ih2, np.float64), np.asarray(b_hh2, np.float64),
        np.asarray(W_lin, np.float64), np.asarray(b_lin, np.float64))
    WG = WG.astype(bf)
    WX = WX.astype(bf)
    xT = np.concatenate([np.ones((1, B), np.float32),
                         input.T.astype(np.float32)]).astype(bf)
    nc = _get_nc(T, Bc, _groups, bool(_ct_f32))
    in_maps = [
        {"xT": np.ascontiguousarray(xT[:, c * Bc:(c + 1) * Bc]),
         "WG": WG, "WX": WX}
        for c in range(N_CORES)
    ]
    res = bass_utils.run_bass_kernel_spmd(
        nc, in_maps, core_ids=list(range(N_CORES)), trace=False)
    outs = [res.results[c]["out_bt"] for c in range(N_CORES)]
    out = np.concatenate(outs, axis=0)
    return out.astype(np.float32)
